# revision 37
# baseline (speedup 1.0000x reference)
"""Trainium2 Bass kernel for BinaryXnorExceptOutliersLinearActivationColumn.

Reference math (all fp32):
    cn[d]    = sum_o |W[o,d]|                         (column L1 norms)
    lower    = quantile(cn, 0.05); upper = quantile(cn, 0.95)   (linear interp)
    mid      = (cn > lower) & (cn < upper)
    mask     = ~mid                                   ("binarized" columns)
    n_bin    = sum(mask)
    scale[o] = sum_d |W[o,d]| * mask[d] / n_bin
    Wb       = where(mask[d], W * scale[o], W)
    out      = x @ Wb.T + bias

Quantile-free mask: with linear interpolation at positions 204.75 / 3890.25
(n=4096), and handling exact ties conservatively,
    mask[d] = (#{j: cn[j] < cn[d]} <= 204) | (#{j: cn[j] > cn[d]} <= 204)
which matches the reference for both distinct values and boundary ties.

Sharding: tensor-parallel over out_features (4096 / 8 = 512 rows per core).
Column norms are computed per-shard and AllReduce'd (16 KB); the rank/mask
computation is replicated on every core. x is replicated; each core computes
out[:, 512c:512(c+1)] and the host concatenates.

On-chip layout: contraction (d) must live on the partition axis for the PE,
so both W-shard and x are transposed on device via PE transpose-mode
(fp32 has no DMA-transpose path).
"""

import os

import numpy as np

import concourse.bass as bass
import concourse.mybir as mybir
import concourse.tile as tile
from concourse import bacc
from concourse.tile_rust import add_dep_helper
from concourse.bass import ts
from concourse.bass_utils import run_bass_kernel_spmd
from concourse.masks import make_identity

P = 128
D_IN = 4096
D_OUT = 4096
N_CORES = 8
O_SH = D_OUT // N_CORES          # 512 out-features per core
KT = D_IN // P                   # 32 contraction tiles
F32 = mybir.dt.float32

# rank thresholds for the 5% / 95% quantile band of 4096 column norms:
# lower in (v[204], v[205]]  -> masked-low  iff #{cn_j < cn_i} <= 204
# upper in [v[3890], v[3891]) -> masked-high iff #{cn_j > cn_i} <= 204
RANK_LO = 204.5
RANK_HI = 204.5

X = mybir.AxisListType.X
C = mybir.AxisListType.C
ALU = mybir.AluOpType


def build(t_rows: int = 8192, mm_dtype: mybir.dt = F32) -> bass.Bass:
    assert t_rows % P == 0
    n_slabs = t_rows // P

    nc = bacc.Bacc(
        "TRN2", target_bir_lowering=False, debug=False, num_devices=N_CORES
    )
    x_d = nc.dram_tensor("x", [t_rows, D_IN], F32, kind="ExternalInput").ap()
    w_d = nc.dram_tensor("w", [O_SH, D_IN], F32, kind="ExternalInput").ap()
    b_d = nc.dram_tensor("b", [1, O_SH], F32, kind="ExternalInput").ap()
    # per-core one-hot selector: sel[k, g] = 1 iff k == 4*core + g
    sel_d = nc.dram_tensor(
        "sel", [KT, KT // N_CORES], F32, kind="ExternalInput"
    ).ap()
    out_d = nc.dram_tensor("out", [t_rows, O_SH], F32, kind="ExternalOutput").ap()

    with tile.TileContext(nc) as tc:
        with (
            tc.tile_pool(name="const", bufs=1) as const_pool,
            tc.tile_pool(name="wtr", bufs=1) as wtr_pool,
            tc.tile_pool(name="keep", bufs=1) as keep_pool,
            tc.tile_pool(name="pst", bufs=4, space="PSUM") as pst_pool,
            tc.tile_pool(name="pso", bufs=2, space="PSUM") as pso_pool,
            tc.tile_pool(name="dram", bufs=1, space="DRAM") as dram_pool,
        ):
            ident = const_pool.tile([P, P], F32, name="ident")
            make_identity(nc, ident)
            if mm_dtype != F32:
                ident_r = const_pool.tile([P, P], mm_dtype, name="ident_r")
                nc.vector.tensor_copy(ident_r, ident)
            else:
                ident_r = ident
            ones_col = const_pool.tile([P, 1], F32, name="ones_col")
            nc.vector.memset(ones_col, 1.0)

            # persistent GEMM weights W_bin^T in mm dtype, one tile per
            # k-slice so the GEMM can start as soon as slice 0 is ready
            wtr = [
                wtr_pool.tile([P, O_SH], mm_dtype, name=f"wtr{k}")
                for k in range(KT)
            ]

            # small persistent tiles
            mask_g = keep_pool.tile([P, KT], F32, name="mask_g")
            scale_m1_bc = keep_pool.tile([P, O_SH], F32, name="scale_m1_bc")
            b_bc = keep_pool.tile([P, O_SH], F32, name="b_bc")

            with tc.tile_pool(name="pre", bufs=1) as pre_pool:
                # full-precision W^T, scoped to preprocessing only
                wt = pre_pool.tile([P, KT, O_SH], F32, name="wt")
                # ---- load W shard (natural layout, half-row chunks),
                # transpose into wt, column norms per half, and a split
                # AllReduce so the first half's collective latency hides
                # under the second half's compute.
                HB = D_IN // 2
                KH = KT // 2
                cn_part = pre_pool.tile([P, KT], F32, name="cn_part")
                cc_outs = [
                    dram_pool.tile([KT // 2, P], F32, addr_space="Shared",
                                   name=f"cc_out{i}")
                    for i in range(2)
                ]
                for dh in range(2):
                    for b4 in range(O_SH // P):
                        w_nat = pre_pool.tile([P, HB], F32, tag="w_nat",
                                              bufs=2, name="w_nat")
                        nc.sync.dma_start(w_nat, w_d[ts(b4, P), ts(dh, HB)])
                        for kk in range(KH):
                            k = dh * KH + kk
                            ps_t = pst_pool.tile([P, P], F32, tag="ps_t",
                                                 name="ps_t")
                            nc.tensor.transpose(ps_t, w_nat[:, ts(kk, P)],
                                                ident)
                            nc.any.tensor_copy(wt[:, k, ts(b4, P)], ps_t)
                    # column norms for this half on ACT (Abs + accum)
                    for kk in range(KH):
                        k = dh * KH + kk
                        mabs0 = pre_pool.tile([P, O_SH], F32, tag="mabs",
                                              bufs=2, name="mabs0")
                        nc.scalar.activation(
                            mabs0, wt[:, k], mybir.ActivationFunctionType.Abs,
                            accum_out=cn_part[:, k : k + 1],
                        )
                    # transpose [128, KH] -> [KH, 128] and AllReduce
                    ps_cnt = pst_pool.tile([KH, P], F32, tag="ps_t",
                                           name="ps_cnt")
                    nc.tensor.transpose(
                        ps_cnt, cn_part[:, ts(dh, KH)], ident
                    )
                    cn_t_sb = pre_pool.tile([KH, P], F32, tag="cn_t_sb",
                                            bufs=1, name="cn_t_sb")
                    nc.vector.tensor_copy(cn_t_sb, ps_cnt)
                    cc_in = dram_pool.tile([KH, P], F32, tag="cc_in", bufs=2,
                                           name="cc_in")
                    nc.gpsimd.dma_start(cc_in, cn_t_sb)
                    nc.gpsimd.collective_compute(
                        "AllReduce",
                        ALU.add,
                        replica_groups=[list(range(N_CORES))],
                        ins=[cc_in.opt()],
                        outs=[cc_outs[dh].opt()],
                    )
                # full cn, flat in d-order, broadcast to all partitions
                cn_bcast = pre_pool.tile([P, D_IN], F32, name="cn_bcast")
                for dh in range(2):
                    nc.sync.dma_start(
                        cn_bcast[0:1, ts(dh, D_IN // 2)],
                        cc_outs[dh].rearrange("a b -> (a b)").unsqueeze(0),
                    )
                nc.gpsimd.partition_broadcast(cn_bcast, cn_bcast[0:1, :])
                # my 4 groups of cn values, selected by the per-core one-hot
                # input: my_cn_g[p, g] = sum_k cn_t[k, p] * sel[k, g]
                n_my = KT // N_CORES
                cn_full_t = pre_pool.tile([KT, P], F32, name="cn_full_t")
                for dh in range(2):
                    nc.sync.dma_start(cn_full_t[ts(dh, KT // 2), :],
                                      cc_outs[dh])
                sel_sb = pre_pool.tile([KT, n_my], F32, name="sel_sb")
                nc.sync.dma_start(sel_sb, sel_d)
                ps_my = pst_pool.tile([P, n_my], F32, tag="ps_t", name="ps_my")
                nc.tensor.matmul(ps_my, cn_full_t, sel_sb)
                my_cn_g = pre_pool.tile([P, n_my], F32, name="my_cn_g")
                nc.vector.tensor_copy(my_cn_g, ps_my)
                neg_my = pre_pool.tile([P, n_my], F32, name="neg_my")
                nc.vector.tensor_scalar(out=neg_my, in0=my_cn_g, scalar1=-1.0,
                                        scalar2=None, op0=ALU.mult)

                # ---- ranks of my 512 columns (sharded across cores).
                # Groups 0-1 on ACT: s = Sign(cn_j - cn_i) with accum
                # (sum_s = cnt_gt - cnt_lt), then Square accum
                # (sum_abs = cnt_gt + cnt_lt). Groups 2-3 on DVE:
                # is_lt / is_gt compares with accum, in two half-width
                # passes to keep the scratch small.
                sum_s = pre_pool.tile([P, n_my], F32, name="sum_s")
                sum_abs = pre_pool.tile([P, n_my], F32, name="sum_abs")
                cnt_lt = pre_pool.tile([P, n_my], F32, name="cnt_lt")
                cnt_gt = pre_pool.tile([P, n_my], F32, name="cnt_gt")
                cl_h = pre_pool.tile([P, 2, 2], F32, name="cl_h")
                cg_h = pre_pool.tile([P, 2, 2], F32, name="cg_h")
                for g in range(2):
                    sg = pre_pool.tile([P, D_IN], F32, tag="sg", bufs=1,
                                       name="sg")
                    nc.scalar.activation(
                        sg, cn_bcast, mybir.ActivationFunctionType.Sign,
                        bias=neg_my[:, g : g + 1],
                        accum_out=sum_s[:, g : g + 1],
                    )
                    nc.scalar.activation(
                        sg, sg, mybir.ActivationFunctionType.Square,
                        accum_out=sum_abs[:, g : g + 1],
                    )
                for g in (2, 3):
                    for h in range(2):
                        junk = pre_pool.tile([P, D_IN // 2], F32, tag="junk",
                                             bufs=1, name="junk")
                        nc.vector.tensor_scalar(
                            out=junk, in0=cn_bcast[:, ts(h, D_IN // 2)],
                            scalar1=my_cn_g[:, g : g + 1],
                            scalar2=None, op0=ALU.is_lt, op1=ALU.add,
                            accum_out=cl_h[:, g - 2, h : h + 1],
                        )
                        junk2 = pre_pool.tile([P, D_IN // 2], F32, tag="junk",
                                              bufs=1, name="junk2")
                        nc.vector.tensor_scalar(
                            out=junk2, in0=cn_bcast[:, ts(h, D_IN // 2)],
                            scalar1=my_cn_g[:, g : g + 1],
                            scalar2=None, op0=ALU.is_gt, op1=ALU.add,
                            accum_out=cg_h[:, g - 2, h : h + 1],
                        )
                # combine: ACT groups -> (sum_abs -+ sum_s)/2 ; DVE groups ->
                # sum of the two half-counts
                nc.vector.tensor_tensor(
                    cnt_lt[:, 0:2], sum_abs[:, 0:2], sum_s[:, 0:2],
                    ALU.subtract,
                )
                nc.vector.tensor_scalar(out=cnt_lt[:, 0:2], in0=cnt_lt[:, 0:2],
                                        scalar1=0.5, scalar2=None,
                                        op0=ALU.mult)
                nc.vector.tensor_tensor(
                    cnt_gt[:, 0:2], sum_abs[:, 0:2], sum_s[:, 0:2], ALU.add
                )
                nc.vector.tensor_scalar(out=cnt_gt[:, 0:2], in0=cnt_gt[:, 0:2],
                                        scalar1=0.5, scalar2=None,
                                        op0=ALU.mult)
                nc.vector.tensor_tensor(
                    cnt_lt[:, 2:4], cl_h[:, :, 0], cl_h[:, :, 1], ALU.add
                )
                nc.vector.tensor_tensor(
                    cnt_gt[:, 2:4], cg_h[:, :, 0], cg_h[:, :, 1], ALU.add
                )

                # mask_my = (cnt_lt <= 204.5) | (cnt_gt <= 204.5), [128, 4]
                m_lo = pre_pool.tile([P, n_my], F32, name="m_lo")
                nc.vector.tensor_scalar(out=m_lo, in0=cnt_lt, scalar1=RANK_LO,
                                        scalar2=None, op0=ALU.is_le)
                m_hi = pre_pool.tile([P, n_my], F32, name="m_hi")
                nc.vector.tensor_scalar(out=m_hi, in0=cnt_gt, scalar1=RANK_HI,
                                        scalar2=None, op0=ALU.is_le)
                mask_my = pre_pool.tile([P, n_my], F32, name="mask_my")
                nc.vector.tensor_tensor(mask_my, m_lo, m_hi, ALU.add)
                nc.vector.tensor_scalar(out=mask_my, in0=mask_my, scalar1=0.5,
                                        scalar2=None, op0=ALU.is_ge)

                # ---- AllGather masks -> mask_g [128, KT] everywhere ----
                ps_mt = pst_pool.tile([n_my, P], F32, tag="ps_t", name="ps_mt")
                nc.tensor.transpose(ps_mt, mask_my, ident)
                mask_my_t = pre_pool.tile([n_my, P], F32, name="mask_my_t")
                nc.vector.tensor_copy(mask_my_t, ps_mt)
                ag_in = dram_pool.tile([n_my, P], F32, name="ag_in")
                nc.gpsimd.dma_start(ag_in, mask_my_t)
                ag_out = dram_pool.tile([KT, P], F32, addr_space="Shared",
                                        name="ag_out")
                nc.gpsimd.collective_compute(
                    "AllGather",
                    ALU.bypass,
                    replica_groups=[list(range(N_CORES))],
                    ins=[ag_in.opt()],
                    outs=[ag_out.opt()],
                )
                mask_t_sb = pre_pool.tile([KT, P], F32, name="mask_t_sb")
                nc.sync.dma_start(mask_t_sb, ag_out)
                ps_mg = pst_pool.tile([P, KT], F32, tag="ps_t", name="ps_mg")
                nc.tensor.transpose(ps_mg, mask_t_sb, ident[:KT, :KT])
                nc.vector.tensor_copy(mask_g, ps_mg)

                # ---- n_bin and 1/n_bin (partition-sum via PE ones-matmul)
                nb_p = pre_pool.tile([P, 1], F32, name="nb_p")
                nc.vector.tensor_reduce(nb_p, mask_g, X, ALU.add)
                ps_nb = pst_pool.tile([1, 1], F32, tag="ps_t", name="ps_nb")
                nc.tensor.matmul(ps_nb, ones_col, nb_p)
                nb = pre_pool.tile([1, 1], F32, name="nb")
                nc.vector.tensor_copy(nb, ps_nb)
                rnb = pre_pool.tile([1, 1], F32, name="rnb")
                nc.vector.reciprocal(rnb, nb)

                # ---- scale[o] = (sum_d |wt| * mask) / n_bin  (PE ones-matmul)
                ps_s = pso_pool.tile([1, O_SH], F32, tag="ps_s", bufs=1,
                                     name="ps_s")
                for k in range(KT):
                    mabs = pre_pool.tile([P, O_SH], F32, tag="mabs", bufs=2,
                                         name="mabs")
                    nc.scalar.activation(
                        mabs, wt[:, k], mybir.ActivationFunctionType.Abs,
                        scale=mask_g[:, k : k + 1],
                    )
                    nc.tensor.matmul(ps_s, ones_col, mabs,
                                     start=(k == 0), stop=(k == KT - 1))
                scale_m1 = pre_pool.tile([1, O_SH], F32, name="scale_m1")
                nc.vector.tensor_scalar(out=scale_m1, in0=ps_s, scalar1=rnb,
                                        scalar2=-1.0, op0=ALU.mult, op1=ALU.add)
                nc.gpsimd.partition_broadcast(scale_m1_bc, scale_m1)

                # ---- apply: wtr[k] = wt * (1 + mask[d] * (scale[o]-1)) ----
                for k in range(KT):
                    fac = pre_pool.tile([P, O_SH], F32, tag="fac", bufs=4,
                                        name="fac")
                    nc.any.tensor_scalar(
                        out=fac, in0=scale_m1_bc,
                        scalar1=mask_g[:, k : k + 1], scalar2=1.0,
                        op0=ALU.mult, op1=ALU.add,
                    )
                    nc.any.tensor_tensor(wtr[k], wt[:, k], fac, ALU.mult)

                # ---- bias broadcast ----
                b_sb = pre_pool.tile([1, O_SH], F32, name="b_sb")
                nc.sync.dma_start(b_sb, b_d)
                nc.gpsimd.partition_broadcast(b_bc, b_sb)

            # ---- main GEMM: out[t, o] = x[t, :] @ wbin[o, :]^T + bias ----
            last_mm = None
            with (
                tc.tile_pool(name="xnat", bufs=3) as xnat_pool,
                tc.tile_pool(name="xt", bufs=3) as xt_pool,
                tc.tile_pool(name="osb", bufs=3) as osb_pool,
            ):
                for i in range(n_slabs):
                    x_nat = xnat_pool.tile([P, D_IN], mm_dtype, tag="x_nat",
                                           name="x_nat")
                    if mm_dtype == F32:
                        nc.sync.dma_start(x_nat, x_d[ts(i, P), :])
                    else:
                        # SWDGE casts (rounds) f32 -> f32r during the load
                        nc.gpsimd.dma_start(x_nat, x_d[ts(i, P), :])
                    xt = xt_pool.tile([P, KT, P], mm_dtype, tag="xt", name="xt")
                    for k in range(KT):
                        ps_t = pst_pool.tile([P, P], mm_dtype, tag="ps_t",
                                             name="ps_t")
                        t_inst = nc.tensor.transpose(
                            ps_t, x_nat[:, ts(k, P)], ident_r
                        )
                        # batch each slab's transposes after the previous
                        # slab's matmuls: an interleaved transpose costs the
                        # following matmul its weight-load pull-ahead (~85ns)
                        if last_mm is not None:
                            add_dep_helper(
                                t_inst.ins, last_mm.ins, sync=False,
                                reason="batch transposes between slab matmuls",
                            )
                        nc.any.tensor_copy(xt[:, k], ps_t)
                    ps_o = pso_pool.tile([P, O_SH], F32, tag="ps_o", bufs=2,
                                         name="ps_o")
                    for k in range(KT):
                        last_mm = nc.tensor.matmul(
                            ps_o, xt[:, k], wtr[k],
                            start=(k == 0), stop=(k == KT - 1),
                        )
                    o_sb = osb_pool.tile([P, O_SH], F32, tag="o_sb",
                                         name="o_sb")
                    nc.vector.tensor_tensor(o_sb, ps_o, b_bc, ALU.add)
                    nc.sync.dma_start(out_d[ts(i, P), :], o_sb)

    nc.compile()
    return nc


_BUILT: dict[tuple, bass.Bass] = {}


def _get_built(t_rows: int, mm_dtype) -> bass.Bass:
    key = (t_rows, str(mm_dtype))
    if key not in _BUILT:
        _BUILT[key] = build(t_rows, mm_dtype)
    return _BUILT[key]


LAST_EXEC_TIME_NS = None


def kernel(x: np.ndarray, weight: np.ndarray, bias: np.ndarray) -> np.ndarray:
    global LAST_EXEC_TIME_NS
    orig_shape = x.shape
    t_rows = int(np.prod(orig_shape[:-1]))
    x2 = np.ascontiguousarray(
        x.reshape(t_rows, D_IN).astype(np.float32, copy=False)
    )
    weight = np.ascontiguousarray(weight.astype(np.float32, copy=False))
    bias = np.ascontiguousarray(bias.astype(np.float32, copy=False))

    mm_dtype_s = os.environ.get("ATH_MM_DTYPE", "f32")
    mm_dtype = {"f32": F32, "f32r": mybir.dt.float32r}[mm_dtype_s]
    trace = os.environ.get("ATH_TRACE", "0") == "1"

    nc = _get_built(t_rows, mm_dtype)

    in_maps = []
    for c in range(N_CORES):
        in_maps.append(
            {
                "x": x2,
                "w": np.ascontiguousarray(weight[c * O_SH : (c + 1) * O_SH]),
                "b": np.ascontiguousarray(bias[c * O_SH : (c + 1) * O_SH])[
                    None, :
                ],
            }
        )

    n_my = KT // N_CORES
    for c in range(N_CORES):
        sel = np.zeros((KT, n_my), dtype=np.float32)
        for g in range(n_my):
            sel[n_my * c + g, g] = 1.0
        in_maps[c]["sel"] = sel

    res = run_bass_kernel_spmd(nc, in_maps, list(range(N_CORES)), trace=trace)
    LAST_EXEC_TIME_NS = res.exec_time_ns

    out = np.concatenate(
        [res.results[c]["out"] for c in range(N_CORES)], axis=1
    )
    return out.reshape(*orig_shape[:-1], D_OUT)
